# revision 20
# baseline (speedup 1.0000x reference)
"""Trainium2 Bass kernel for ModLinear forward:

    alpha = z @ weight_alpha.T + bias_alpha          # [B, IN]
    beta  = z @ weight_beta.T  + bias_beta           # [B, OUT]
    out   = (x * alpha[:, None, :]) @ weight.T + beta[:, None, :]

Restructuring: alpha modulates input channels, so it folds into the weight
per batch: out[b] = x[b] @ (weight.T * alpha[b][:, None]) + beta[b]. The
huge x tensor is consumed by a plain matmul with a tiny per-batch
pre-modulated weight (computed on host along with alpha/beta).

This version halves HBM traffic vs the fp32 baseline by running the matmul
in bf16 (x, w_mod, out all bf16; accumulation stays fp32 in PSUM; beta is
added in fp32 before the bf16 round). Quantization error ~3.9e-3 vs the
2e-2 gate. x is also pre-TRANSPOSED on host to [IN, rows] per core, so the
contraction dim lands on SBUF partitions straight from DMA and the PE does
no transposes at all — the kernel becomes PE-streaming-bound:

  per core: 1024 back-to-back bf16 matmuls [128x128] @ [128x512],
  measured 215 ns each (512 cols @ 2.4 GHz + NX overhead) in good clock
  windows (the chip's PE PLL drops to ~2.0 GHz under some machine power
  states — run-to-run variance), with 32+32 MiB of DMA (~187 us at the
  358 GB/s per-core HBM cap) hidden underneath. Measured best ~245 us.

Sharding: rows = B*N flattened, 8 contiguous 32768-row blocks, one per
core; batch boundary falls between cores 3 and 4. No cross-core comms.

Device kernel per core, superblocks of 4096 rows (columns of xT):
  prologue: 14 dummy matmuls on a memset tile warm the PE HAM clock gate
    while the first loads run. wp (bf16 weights, oc-major packed) is
    quartered across both HWDGE rings; after the fixed ~7us NEFF startup
    barrier the head is dispatch-serialization-bound (~0.65us per
    dma_start per ring), so the first superblock loads in [1024, 1024,
    2048]-column waves split ic0/1 -> sync ring, ic2/3 -> scalar ring.
  for each superblock s (input prefetched 2 deep, 4x 1 MiB on sync):
    for jj in 4 (1024-col groups; jj-major so the PE consumes columns in
                 DMA-arrival order and never stalls/re-throttles):
      for oc in 4:
        8 matmuls accumulate 4 ic-chunks x 2 512-col halves into a
        [128,1024] PSUM tile (2 banks, pool bufs=4 -> all 8 banks)
        DVE tensor_scalar_add: PSUM + beta[oc] -> bf16 SBUF
        store 2048-col pieces (1024 on the last superblock to shrink
        the pipeline tail) on the scalar ring
Host: un-transpose + fp32-cast the gathered outT blocks.
"""

import numpy as np

B, N = 2, 131072
IN_F, OUT_F, STYLE_F = 512, 512, 256
NCORES = 8
ROWS = B * N
ROWS_PER_CORE = ROWS // NCORES  # 32768
P = 128
SUP = 4096  # columns (rows of x) per superblock
NSUP = ROWS_PER_CORE // SUP  # 8
NB2 = SUP // 1024  # 1024-col psum groups per superblock


def _build_body(tc, outt_ap, xt_ap, wp_ap, betac_ap):
    import concourse.bass as bass
    from concourse import mybir

    nc = tc.nc
    f32 = mybir.dt.float32
    bf16 = mybir.dt.bfloat16

    with (
        tc.tile_pool(name="const", bufs=1) as cpool,
        tc.tile_pool(name="xin", bufs=3) as xpool,
        tc.tile_pool(name="oout", bufs=3) as opool,
        tc.tile_pool(name="pmm", bufs=4, space="PSUM") as pmpool,
    ):
        # Warmup weights: memset (no DMA dependency) so the PE can start
        # warming its HAM clock gate immediately, before any data lands.
        wz_sb = cpool.tile([P, 512], bf16)
        nc.vector.memset(wz_sb[:], 0.125)
        po_warm = pmpool.tile([P, 1024], f32, name="po", tag="po")
        for _ in range(14):
            nc.tensor.matmul(
                po_warm[:, :512], wz_sb[:, :P], wz_sb[:],
                start=True, stop=True,
            )

        def load_xt(xt, s, splits, engines=(None, None, None, None)):
            c0 = 0
            for w in splits:
                for ic in range(4):
                    eng = engines[ic] or nc.sync
                    eng.dma_start(
                        out=xt[:, ic * SUP + c0 : ic * SUP + c0 + w],
                        in_=xt_ap[
                            ic * P : (ic + 1) * P,
                            s * SUP + c0 : s * SUP + c0 + w,
                        ],
                    )
                c0 += w

        # Weights first (every matmul needs them), oc-major packed and
        # quartered across BOTH HWDGE rings — after the fixed ~7us NEFF
        # startup barrier the head is gated by the ~0.65us per-dma
        # dispatch serialization on each ring, so dispatch counts before
        # the first compute groups are what matter.
        wp_sb = cpool.tile([P, 16 * P], bf16)
        wp_eng = (nc.sync, nc.scalar, nc.sync, nc.scalar)
        for oc in range(4):
            wp_eng[oc].dma_start(
                out=wp_sb[:, oc * 4 * P : (oc + 1) * 4 * P],
                in_=wp_ap[:, oc * 4 * P : (oc + 1) * 4 * P],
            )
        betac_sb = cpool.tile([P, 4], f32)
        nc.scalar.dma_start(out=betac_sb[:], in_=betac_ap[:, :])
        head_eng = (nc.sync, nc.sync, nc.scalar, nc.scalar)
        xt0 = xpool.tile([P, 4 * SUP], bf16, name="xt", tag="xt")
        load_xt(xt0, 0, [1024, 1024, 2048], head_eng)
        xt1 = xpool.tile([P, 4 * SUP], bf16, name="xt", tag="xt")
        load_xt(xt1, 1, [SUP], head_eng)
        xts = [xt0, xt1]

        for s in range(NSUP):
            # 2-deep input prefetch.
            if s + 2 < NSUP:
                xtn = xpool.tile([P, 4 * SUP], bf16, name="xt", tag="xt")
                load_xt(xtn, s + 2, [SUP])
                xts.append(xtn)
            xt = xts[s]
            ot = opool.tile([P, 4 * SUP], bf16)

            # jj-major: consume x columns strictly in DMA-arrival order so
            # the PE never chases the tail of the input stream (which would
            # stall it and re-throttle the HAM clock gate).
            for jj in range(NB2):
                for oc in range(4):
                    po = pmpool.tile([P, 1024], f32, name="po", tag="po")
                    for ic in range(4):
                        w_ch = wp_sb[:, (oc * 4 + ic) * P : (oc * 4 + ic + 1) * P]
                        for g in range(2):
                            col0 = jj * 1024 + g * 512
                            nc.tensor.matmul(
                                po[:, g * 512 : (g + 1) * 512],
                                w_ch,
                                xt[:, ic * SUP + col0 : ic * SUP + col0 + 512],
                                start=(ic == 0),
                                stop=(ic == 3),
                            )
                    last_piece = s == NSUP - 1 and jj == NB2 - 1 and oc == 3
                    if last_piece:
                        # Very last piece: split drain+store in half so the
                        # pipeline tail after the final matmul is minimal.
                        for g in range(2):
                            nc.vector.tensor_scalar_add(
                                out=ot[
                                    :,
                                    oc * SUP + jj * 1024 + g * 512 :
                                    oc * SUP + jj * 1024 + (g + 1) * 512,
                                ],
                                in0=po[:, g * 512 : (g + 1) * 512],
                                scalar1=betac_sb[:, oc : oc + 1],
                            )
                            nc.scalar.dma_start(
                                out=outt_ap[
                                    oc * P : (oc + 1) * P,
                                    s * SUP + jj * 1024 + g * 512 :
                                    s * SUP + jj * 1024 + (g + 1) * 512,
                                ],
                                in_=ot[
                                    :,
                                    oc * SUP + jj * 1024 + g * 512 :
                                    oc * SUP + jj * 1024 + (g + 1) * 512,
                                ],
                            )
                        continue
                    nc.vector.tensor_scalar_add(
                        out=ot[:, oc * SUP + jj * 1024 : oc * SUP + (jj + 1) * 1024],
                        in0=po[:],
                        scalar1=betac_sb[:, oc : oc + 1],
                    )
                    # Stores: 2048-col pieces in steady state (halve the
                    # ~0.6us/dma dispatch load on the scalar ring); finer
                    # 1024-col pieces on the last superblock to minimize
                    # the pipeline tail.
                    if s == NSUP - 1:
                        nc.scalar.dma_start(
                            out=outt_ap[
                                oc * P : (oc + 1) * P,
                                s * SUP + jj * 1024 : s * SUP + (jj + 1) * 1024,
                            ],
                            in_=ot[
                                :, oc * SUP + jj * 1024 : oc * SUP + (jj + 1) * 1024
                            ],
                        )
                    elif jj % 2 == 1:
                        h0 = (jj - 1) * 1024
                        nc.scalar.dma_start(
                            out=outt_ap[
                                oc * P : (oc + 1) * P,
                                s * SUP + h0 : s * SUP + h0 + 2048,
                            ],
                            in_=ot[:, oc * SUP + h0 : oc * SUP + h0 + 2048],
                        )


def build_nc(rows_per_core=ROWS_PER_CORE):
    """Build + compile the per-core Bass program. Returns nc."""
    import concourse.tile as tile
    from concourse import bacc, mybir

    f32 = mybir.dt.float32
    bf16 = mybir.dt.bfloat16
    nc = bacc.Bacc(
        "TRN2", target_bir_lowering=False, debug=False, num_devices=NCORES
    )
    xt_t = nc.dram_tensor("xt", [IN_F, rows_per_core], bf16, kind="ExternalInput")
    wp_t = nc.dram_tensor("wp", [P, 16 * P], bf16, kind="ExternalInput")
    betac_t = nc.dram_tensor("betac", [P, 4], f32, kind="ExternalInput")
    outt_t = nc.dram_tensor(
        "outt", [OUT_F, rows_per_core], bf16, kind="ExternalOutput"
    )

    with tile.TileContext(nc) as tc:
        _build_body(tc, outt_t.ap(), xt_t.ap(), wp_t.ap(), betac_t.ap())
    nc.compile()
    return nc


_NC_CACHE = {}


def _get_nc(rows_per_core=ROWS_PER_CORE):
    if rows_per_core not in _NC_CACHE:
        _NC_CACHE[rows_per_core] = build_nc(rows_per_core)
    return _NC_CACHE[rows_per_core]


def host_prep(x, z, weight, weight_alpha, bias_alpha, weight_beta, bias_beta):
    """Per-batch modulated weights + biases in device layout; per-core
    bf16 transposed x shards."""
    import ml_dtypes

    bf16 = np.dtype(ml_dtypes.bfloat16)

    z64 = z.astype(np.float64)
    alpha = (z64 @ weight_alpha.astype(np.float64).T) + bias_alpha.astype(np.float64)
    beta = (z64 @ weight_beta.astype(np.float64).T) + bias_beta.astype(np.float64)
    alpha = alpha.astype(np.float32)  # [B, IN_F]
    beta = beta.astype(np.float32)  # [B, OUT_F]

    wps = []
    betacs = []
    for b in range(B):
        wm = (weight.T * alpha[b][:, None]).astype(bf16)  # [IN, OUT]
        # wp[p, (oc*4+ic)*128 + m] = wm[ic*128+p, oc*128+m]
        wp = np.ascontiguousarray(
            wm.reshape(4, P, 4, P).transpose(1, 2, 0, 3).reshape(P, 16 * P)
        )
        wps.append(wp)
        betacs.append(np.ascontiguousarray(beta[b].reshape(4, P).T))  # [128, 4]

    xb = np.ascontiguousarray(x).reshape(ROWS, IN_F).astype(bf16)
    in_maps = []
    for k in range(NCORES):
        b = (k * ROWS_PER_CORE) // N
        xs = xb[k * ROWS_PER_CORE : (k + 1) * ROWS_PER_CORE]  # [rows, IN]
        in_maps.append(
            {
                "xt": np.ascontiguousarray(xs.T),  # [IN, rows] bf16
                "wp": wps[b],
                "betac": betacs[b],
            }
        )
    return in_maps


def kernel(x, z, weight, weight_alpha, bias_alpha, weight_beta, bias_beta,
           _trace=False):
    from concourse.bass_utils import run_bass_kernel_spmd

    x = np.asarray(x, dtype=np.float32)
    z = np.asarray(z, dtype=np.float32)
    weight = np.asarray(weight, dtype=np.float32)
    weight_alpha = np.asarray(weight_alpha, dtype=np.float32)
    bias_alpha = np.asarray(bias_alpha, dtype=np.float32)
    weight_beta = np.asarray(weight_beta, dtype=np.float32)
    bias_beta = np.asarray(bias_beta, dtype=np.float32)
    in_maps = host_prep(
        x, z, weight, weight_alpha, bias_alpha, weight_beta, bias_beta
    )
    nc = _get_nc()
    res = run_bass_kernel_spmd(
        nc, in_maps, core_ids=list(range(NCORES)), trace=_trace
    )
    # Gather: outt [OUT, rows] bf16 per core -> [rows, OUT] fp32 full.
    out = np.empty((ROWS, OUT_F), dtype=np.float32)
    for k in range(NCORES):
        blk = np.asarray(res.results[k]["outt"])  # [OUT, rpc] bf16
        out[k * ROWS_PER_CORE : (k + 1) * ROWS_PER_CORE] = blk.T.astype(np.float32)
    out = out.reshape(B, N, OUT_F)
    if _trace:
        kernel.last_results = res
    return out
